# revision 1
# baseline (speedup 1.0000x reference)
"""LocalSelfAttention kernel, 8-way spatially sharded over H with 1-row halo.

Shapes hardcoded per spec: x [2,256,96,96], w_qkv [768,256], w_out [256,256],
b_out [256]. Each of the 8 cores computes 12 output rows; the k=3 unfold
needs 1 halo row on each side, provided by overlapped input slicing (zero
rows at the image boundary reproduce the reference's zero padding exactly,
since the qkv projection has no bias).
"""
import numpy as np

HEADS = 8
KSIZE = 3
B, C, H, W = 2, 256, 96, 96
NCORES = 8
ROWS = H // NCORES  # 12 output rows per core

_pfn = None


def _build_pfn():
    import jax
    import jax.numpy as jnp

    hd = C // HEADS
    kk = KSIZE * KSIZE
    scale = hd ** (-0.5)

    def shard_fn(xs, w_qkv, w_out, b_out):
        # xs: [B, C, ROWS+2, W] -- one halo row above and below
        qkv = jnp.einsum('bchw,oc->bohw', xs, w_qkv)
        q, k, v = jnp.split(qkv, 3, axis=1)

        def unfold_center(t):
            # zero-pad W only; H halo comes from the overlapped slice
            tp = jnp.pad(t, ((0, 0), (0, 0), (0, 0), (1, 1)))
            pats = [tp[:, :, di:di + ROWS, dj:dj + W]
                    for di in range(KSIZE) for dj in range(KSIZE)]
            return jnp.stack(pats, axis=2)  # [B, C, 9, ROWS, W]

        q, k, v = [unfold_center(t).reshape(B, HEADS, hd, kk, ROWS, W)
                   for t in (q, k, v)]
        dots = jnp.einsum('bhnsij,bhmsij->bhnmij', q * scale, k)
        attn = jax.nn.softmax(dots, axis=3)
        out = jnp.einsum('bhnmij,bhmsij->bhnsij', attn, v)
        out = out.reshape(B, C, kk, ROWS, W).sum(axis=2)
        out = (jnp.einsum('bchw,oc->bohw', out, w_out)
               + b_out[None, :, None, None] + xs[:, :, 1:1 + ROWS, :])
        return out

    return jax.pmap(shard_fn)


def _kernel_numpy(x, w_qkv, w_out, b_out):
    # CPU fallback, exact reference math
    hd = C // HEADS
    kk = KSIZE * KSIZE
    scale = hd ** (-0.5)
    qkv = np.einsum('bchw,oc->bohw', x, w_qkv)
    q, k, v = np.split(qkv, 3, axis=1)

    def unfold(t):
        tp = np.pad(t, ((0, 0), (0, 0), (1, 1), (1, 1)))
        pats = [tp[:, :, i:i + H, j:j + W] for i in range(KSIZE) for j in range(KSIZE)]
        return np.stack(pats, axis=2)

    q, k, v = [unfold(t).reshape(B, HEADS, hd, kk, H, W) for t in (q, k, v)]
    dots = np.einsum('bhnsij,bhmsij->bhnmij', q * scale, k)
    dots -= dots.max(axis=3, keepdims=True)
    e = np.exp(dots)
    attn = e / e.sum(axis=3, keepdims=True)
    out = np.einsum('bhnmij,bhmsij->bhnsij', attn, v)
    out = out.reshape(B, C, kk, H, W).sum(axis=2)
    out = np.einsum('bchw,oc->bohw', out, w_out) + b_out[None, :, None, None] + x
    return out.astype(np.float32)


def kernel(x, w_qkv, w_out, b_out):
    global _pfn
    x = np.asarray(x, np.float32)
    w_qkv = np.asarray(w_qkv, np.float32)
    w_out = np.asarray(w_out, np.float32)
    b_out = np.asarray(b_out, np.float32)
    try:
        import jax
        if len(jax.devices()) < NCORES:
            raise RuntimeError('fewer than 8 devices')
        if _pfn is None:
            _pfn = _build_pfn()
        xp = np.zeros((B, C, H + 2, W), np.float32)
        xp[:, :, 1:H + 1, :] = x
        shards = np.stack([xp[:, :, r * ROWS:r * ROWS + ROWS + 2, :]
                           for r in range(NCORES)])
        wq = np.broadcast_to(w_qkv, (NCORES,) + w_qkv.shape)
        wo = np.broadcast_to(w_out, (NCORES,) + w_out.shape)
        bo = np.broadcast_to(b_out, (NCORES,) + b_out.shape)
        out = np.asarray(_pfn(shards, wq, wo, bo))  # [8, B, C, ROWS, W]
        return np.concatenate(list(out), axis=2).astype(np.float32)
    except Exception:
        return _kernel_numpy(x, w_qkv, w_out, b_out)

